# revision 14
# baseline (speedup 1.0000x reference)
"""3-layer GCN (gcn_norm + 3x gcn_conv + softmax) on 8 Trainium2 NeuronCores.

Strategy (self-contained; shapes hardcoded for N=16384, E=524288):
  - Node (row) sharding: core d owns nodes [d*2048, (d+1)*2048).
  - Everything big is fp8-e4m3 with power-of-2 scale factors folded into the
    per-node scale tensors, enabling the PE's DoubleRow perf mode (2 k-tiles
    of 128 per pass) for both the x@W1 stage and the dense aggregations.
  - Stage 1: P1 = x @ (W1*64) from a host-transposed fp8 x shard; the 1/64
    is folded into the dis scale applied afterwards.
  - Aggregation out = D^-1/2 (A + I) D^-1/2 (h W): normalization folded into
    per-node scales dis = deg^-1/2 applied before/after a plain *count*
    aggregation.  The count matrix (A + I) is dense fp8 per-core column shard
    [16384, 2048]; small-integer counts are exact in fp8.
  - The first 48 source-tiles of the count matrix (12 MiB) stay RESIDENT in
    SBUF across all three layers; only the remaining 20 MiB is re-streamed
    per layer.  Each layer consumes the resident part first so the PE can
    start right after the AllGather while the stream refills.
  - Features are stored as alpha_l * dis_n * (h W)_n in fp8; the epilogue
    multiplies by dis_t (one shared disrep) with 1/alpha_l folded into the
    activation's scale argument.
  - Per-core node shards of the fp8 features are AllGathered so every core
    holds the full source-side operand in SBUF.
  - Input DRAM tensors are declared in consumption order and the big ones
    split, so the runtime's input upload pipelines with compute.

kernel(**inputs) takes the FULL inputs and returns the FULL [16384, 16] fp32
output.
"""

import numpy as np

import concourse.bass as bass
import concourse.mybir as mybir
import concourse.tile as tile
from concourse import bacc
from concourse.bass_utils import run_bass_kernel_spmd
from concourse.masks import make_identity

N = 16384
NCORES = 8
CP = N // NCORES          # 2048 nodes per core
F1, F2, F3 = 64, 32, 16
KT = N // 128             # 128 k-tiles in stage 1
ST = N // 128             # 128 source tiles in aggregation
MT = CP // 128            # 16 m-tiles (local node tiles)
NCHUNK = CP // 512        # 4 free-dim chunks of 512
SB = 8                    # k/source tiles per 2 MiB stream DMA
RES = 40                  # resident source tiles (10 MiB SBUF)
NXT = 4                   # xt_d split into NXT input tensors

W1_SCALE = 64.0           # W1 pre-scaled into e4m3 normal range
ALPHA = {1: 4.0, 2: 32.0, 3: 128.0}   # fp8 feature scales per layer

F32 = mybir.dt.float32
F16 = mybir.dt.float16
FP8 = mybir.dt.float8e4
NP_FP8 = mybir.dt.np(FP8)
U8 = mybir.dt.uint8
DR = mybir.MatmulPerfMode.DoubleRow

_prog_cache = {}


def _build_program():
    nc = bacc.Bacc("TRN2", target_bir_lowering=False, debug=False,
                   num_devices=NCORES)

    # Declaration order == consumption order (the runtime's input upload is
    # roughly in-order; the kernel's first reads then wait minimally).
    w1_d = nc.dram_tensor("w1_d", [128, KT * F1], FP8, kind="ExternalInput")
    w2_d = nc.dram_tensor("w2_d", [F1, F2], F32, kind="ExternalInput")
    w3_d = nc.dram_tensor("w3_d", [F2, F3], F32, kind="ExternalInput")
    b1_d = nc.dram_tensor("b1_d", [F1, 1], F32, kind="ExternalInput")
    b2_d = nc.dram_tensor("b2_d", [F2, 1], F32, kind="ExternalInput")
    b3_d = nc.dram_tensor("b3_d", [F3, 1], F32, kind="ExternalInput")
    dis16_l1 = nc.dram_tensor("dis16_l1", [128, MT], F32, kind="ExternalInput")
    dis16_l2 = nc.dram_tensor("dis16_l2", [128, MT], F32, kind="ExternalInput")
    dis16_l3 = nc.dram_tensor("dis16_l3", [128, MT], F32, kind="ExternalInput")
    disrep_d = nc.dram_tensor("disrep_d", [F1, CP], F32, kind="ExternalInput")
    # xt parts: [p, kb, b, t], feature = ((x*(KT//SB//NXT)+kb)*8+b)*128 + p
    xt_d = [nc.dram_tensor(f"xt{i}_d", [128, N * CP // 128 // NXT], FP8,
                           kind="ExternalInput") for i in range(NXT)]
    # resident part of the count matrix: source tiles 0..RES-1
    ares_d = nc.dram_tensor("ares_d", [128, RES * CP], FP8,
                            kind="ExternalInput")
    # streamed part: source tiles RES..127 as a 1-bit indicator bitmask
    # packed along t (8 targets/byte); expanded on-device to fp8 0/1.
    # (the ~48 entries/core with count>=2 are clipped to 1 -- negligible)
    astp_d = nc.dram_tensor("astp_d", [128, (ST - RES) * CP // 8], U8,
                            kind="ExternalInput")
    astr_x = nc.dram_tensor("astr_x", [128, (ST - RES) * CP], U8)
    out_d = nc.dram_tensor("out_d", [CP, F3], F32, kind="ExternalOutput")

    # DRAM bounce buffers for the AllGathers (in: Local, out: Shared)
    ps_in = {}
    ps_out = {}
    for li, F in ((1, F1), (2, F2), (3, F3)):
        ps_in[li] = nc.dram_tensor(f"ps_in_{li}", [CP, F], FP8)
        ps_out[li] = nc.dram_tensor(f"ps_out_{li}", [N, F], FP8,
                                    addr_space="Shared")

    with tile.TileContext(nc) as tc:
        with tc.tile_pool(name="const", bufs=1) as cpool, \
             tc.tile_pool(name="stream", bufs=2) as spool, \
             tc.tile_pool(name="ps", bufs=1) as pspool, \
             tc.tile_pool(name="xpand", bufs=2) as xpool, \
             tc.tile_pool(name="work", bufs=1) as wpool, \
             tc.tile_pool(name="psum", bufs=2, space="PSUM") as psum, \
             tc.tile_pool(name="psum_acc", bufs=1, space="PSUM") as psum_acc:

            # ---- constants -------------------------------------------------
            w1_sb = cpool.tile([128, KT * F1], FP8, tag="w1")
            nc.scalar.dma_start(out=w1_sb[:], in_=w1_d[:, :])
            w2_sb = cpool.tile([F1, F2], F32, tag="w2")
            nc.scalar.dma_start(out=w2_sb[:], in_=w2_d[:, :])
            w3_sb = cpool.tile([F2, F3], F32, tag="w3")
            nc.scalar.dma_start(out=w3_sb[:], in_=w3_d[:, :])
            b_sb = {}
            for li, (bd, F) in ((1, (b1_d, F1)), (2, (b2_d, F2)), (3, (b3_d, F3))):
                b_sb[li] = cpool.tile([F, 1], F32, tag=f"b{li}", name=f"b{li}_sb")
                nc.scalar.dma_start(out=b_sb[li][:], in_=bd[:, :])
            dis16_sb = {}
            for li, dd in ((1, dis16_l1), (2, dis16_l2), (3, dis16_l3)):
                dis16_sb[li] = cpool.tile([128, MT], F32, tag=f"dis16_{li}",
                                          name=f"dis16_sb{li}")
                nc.scalar.dma_start(out=dis16_sb[li][:], in_=dd[:, :])
            disrep_sb = cpool.tile([F1, CP], F32, tag="disrep")
            nc.scalar.dma_start(out=disrep_sb[:], in_=disrep_d[:, :])
            ident = cpool.tile([128, 128], F32, tag="ident")
            make_identity(nc, ident[:])

            # resident count-matrix block, loaded once (6 x 2 MiB DMAs)
            ares_sb = cpool.tile([128, RES, CP], FP8, tag="ares")
            aresr = ares_sb[:]

            def strided_m(t, m):
                """[F, CP] tile -> [F, 128] slice holding nodes q*16+m."""
                return t[:].rearrange("f (q m) -> f m q", m=MT)[:, m, :]

            rings = [nc.sync, nc.scalar, nc.gpsimd]

            # ---- stage 1: P1T = (x @ W1)^T for local nodes ----------------
            w1r = w1_sb[:].rearrange("p (k f) -> p k f", k=KT)
            p1t_ps = [psum_acc.tile([F1, 512], F32, tag=f"acc{c}",
                                    name=f"p1t_ps{c}")
                      for c in range(NCHUNK)]
            KBT = KT // SB // NXT   # kb tiles per xt part
            for kb in range(KT // SB):
                xt_tile = spool.tile([128, SB, CP], FP8, tag="stream")
                off = (kb % KBT) * SB * CP
                rings[kb % 2].dma_start(
                    out=xt_tile[:],
                    in_=xt_d[kb // KBT][:, off:off + SB * CP]
                        .rearrange("p (b t) -> p b t", b=SB))
                # interleave the resident-A load on the third ring
                if kb % 2 == 0 and kb // 2 < RES // SB:
                    jb = kb // 2
                    nc.gpsimd.dma_start(
                        out=ares_sb[:, jb * SB:(jb + 1) * SB, :],
                        in_=ares_d[:, jb * SB * CP:(jb + 1) * SB * CP]
                            .rearrange("p (b t) -> p b t", b=SB))
                for bp in range(SB // 2):
                    k2 = kb * (SB // 2) + bp       # pair index, 0..63
                    for c in range(NCHUNK):
                        nc.tensor.matmul(
                            p1t_ps[c][:],
                            lhsT=w1r[:, 2 * k2:2 * k2 + 2, :],
                            rhs=xt_tile[:, 2 * bp:2 * bp + 2,
                                        c * 512:(c + 1) * 512],
                            start=(k2 == 0), stop=(k2 == KT // 2 - 1),
                            perf_mode=DR)
            # expand the streamed count-matrix bitmask to fp8 0/1 in DRAM
            # (DVE+GpSimd are idle during stage 1; writes/re-reads of astr_x
            # are cheap -- only the upload bytes are expensive)
            for xb in range((ST - RES) // SB):
                apk = xpool.tile([128, SB, CP // 8], U8, tag="apk")
                nc.gpsimd.dma_start(
                    out=apk[:],
                    in_=astp_d[:, xb * SB * CP // 8:(xb + 1) * SB * CP // 8]
                        .rearrange("p (b t) -> p b t", b=SB))
                xo = xpool.tile([128, SB, CP], U8, tag="xo")
                xor_ = xo[:].rearrange("p b (t8 j) -> p b t8 j", j=8)
                for j in range(8):
                    eng = nc.vector  # bitVec ops are DVE-only
                    # bitVec ops cannot cast: stay u8, park the bit at
                    # position 3 so the byte is 0x08 == fp8 2^-6 (the
                    # resident counts are pre-scaled by 2^-6 to match;
                    # the epilogue multiplies by 64)
                    if j <= 3:
                        eng.tensor_scalar(
                            xor_[:, :, :, j], apk[:], 3 - j, 0x08,
                            mybir.AluOpType.logical_shift_left,
                            mybir.AluOpType.bitwise_and)
                    else:
                        eng.tensor_scalar(
                            xor_[:, :, :, j], apk[:], j - 3, 0x08,
                            mybir.AluOpType.logical_shift_right,
                            mybir.AluOpType.bitwise_and)
                nc.gpsimd.dma_start(
                    out=astr_x[:, xb * SB * CP:(xb + 1) * SB * CP]
                        .rearrange("p (b t) -> p b t", b=SB),
                    in_=xo[:])

            p1t_sb = wpool.tile([F1, CP], F32, tag="hT")
            for c in range(NCHUNK):
                nc.vector.tensor_copy(p1t_sb[:, c * 512:(c + 1) * 512],
                                      p1t_ps[c][:])
            ps_local1 = wpool.tile([128, MT, F1], FP8, tag="psl1")
            for m in range(MT):
                pt = psum.tile([128, F1], F32, tag="wmul")
                nc.tensor.transpose(pt[:], strided_m(p1t_sb, m),
                                    ident[:F1, :F1])
                nc.vector.tensor_scalar_mul(ps_local1[:, m, :], pt[:],
                                            dis16_sb[1][:, m:m + 1])
            nc.sync.dma_start(
                out=ps_in[1].ap().rearrange("(p m) f -> p m f", p=128),
                in_=ps_local1[:])
            nc.gpsimd.collective_compute(
                "AllGather", mybir.AluOpType.bypass,
                replica_groups=[list(range(NCORES))],
                ins=[ps_in[1].ap().opt()],
                outs=[ps_out[1].ap().opt()],
            )

            def emit_wmul_scale_gather(hT_sb, F_in, F_nxt, w_sb, li):
                """normal-land W-mul + dis scale + fp8 cast, per m-tile with
                node-to-(partition, m) mapping n = p*16 + m; then bounce to
                DRAM and AllGather."""
                ps_local = wpool.tile([128, MT, F_nxt], FP8, tag=f"psl{li}")
                for m in range(MT):
                    pt = psum.tile([128, F_nxt], F32, tag="wmul")
                    nc.tensor.matmul(pt[:], lhsT=strided_m(hT_sb, m),
                                     rhs=w_sb[:], start=True, stop=True)
                    nc.vector.tensor_scalar_mul(ps_local[:, m, :], pt[:],
                                                dis16_sb[li][:, m:m + 1])
                nc.sync.dma_start(
                    out=ps_in[li].ap().rearrange("(p m) f -> p m f", p=128),
                    in_=ps_local[:])
                nc.gpsimd.collective_compute(
                    "AllGather", mybir.AluOpType.bypass,
                    replica_groups=[list(range(NCORES))],
                    ins=[ps_in[li].ap().opt()],
                    outs=[ps_out[li].ap().opt()],
                )
                ps_full = pspool.tile([128, ST * F_nxt], FP8, tag=f"psf{li}")
                # dram row of (p, d, m) is d*2048 + p*16 + m; j = d*16 + m
                nc.sync.dma_start(
                    out=ps_full[:].rearrange("p (d m f) -> p d m f",
                                             d=NCORES, m=MT),
                    in_=ps_out[li].ap().rearrange("(d p m) f -> p d m f",
                                                  d=NCORES, p=128))
                return ps_full

            ps_full = pspool.tile([128, ST * F1], FP8, tag="psf1")
            nc.scalar.dma_start(
                out=ps_full[:].rearrange("p (d m f) -> p d m f",
                                         d=NCORES, m=MT),
                in_=ps_out[1].ap().rearrange("(d p m) f -> p d m f",
                                             d=NCORES, p=128))

            # ---- layers ----------------------------------------------------
            for li, F in ((1, F1), (2, F2), (3, F3)):
                psfr = ps_full[:].rearrange("p (j f) -> p j f", j=ST)
                agg_ps = [psum_acc.tile([F, 512], F32, tag=f"acc{c}",
                                        name=f"agg{li}_ps{c}")
                          for c in range(NCHUNK)]

                def pair_mm(j2, rhs_ap, start, stop):
                    for c in range(NCHUNK):
                        nc.tensor.matmul(
                            agg_ps[c][:],
                            lhsT=psfr[:, 2 * j2:2 * j2 + 2, :],
                            rhs=rhs_ap[:, :, c * 512:(c + 1) * 512],
                            start=start, stop=stop, perf_mode=DR)

                # resident source tiles first: PE starts right after the
                # AllGather while the stream part is still in flight
                for j2 in range(RES // 2):
                    pair_mm(j2, aresr[:, 2 * j2:2 * j2 + 2, :],
                            start=(j2 == 0), stop=False)
                # streamed source tiles RES..127
                for jb in range((ST - RES) // SB):
                    a_tile = spool.tile([128, SB, CP], U8, tag="stream")
                    eng = rings[jb % 3]
                    eng.dma_start(
                        out=a_tile[:],
                        in_=astr_x[:, jb * SB * CP:(jb + 1) * SB * CP]
                            .rearrange("p (b t) -> p b t", b=SB))
                    for bp in range(SB // 2):
                        j2 = RES // 2 + jb * (SB // 2) + bp
                        pair_mm(j2,
                                a_tile[:, 2 * bp:2 * bp + 2, :].bitcast(FP8),
                                start=False, stop=(j2 == ST // 2 - 1))

                hT_sb = wpool.tile([F, CP], F32, tag="hT")
                for c in range(NCHUNK):
                    sl = slice(c * 512, (c + 1) * 512)
                    zt = wpool.tile([F, 512], F32, tag="zt")
                    nc.vector.tensor_tensor(zt[:], agg_ps[c][:],
                                            disrep_sb[:F, sl],
                                            mybir.AluOpType.mult)
                    func = (mybir.ActivationFunctionType.Relu if li < 3
                            else mybir.ActivationFunctionType.Identity)
                    nc.scalar.activation(hT_sb[:, sl], zt[:], func,
                                         bias=b_sb[li][:, 0:1],
                                         scale=64.0 / ALPHA[li])
                if li == 1:
                    ps_full = emit_wmul_scale_gather(hT_sb, F1, F2, w2_sb, 2)
                elif li == 2:
                    ps_full = emit_wmul_scale_gather(hT_sb, F2, F3, w3_sb, 3)
                else:
                    # transpose + batched softmax over classes (free dim)
                    h3 = wpool.tile([128, MT, F3], F32, tag="h3")
                    for m in range(MT):
                        pt = psum.tile([128, F3], F32, tag="wmul")
                        nc.tensor.transpose(pt[:], strided_m(hT_sb, m),
                                            ident[:F3, :F3])
                        nc.vector.tensor_copy(h3[:, m, :], pt[:])
                    mx = wpool.tile([128, MT], F32, tag="mx")
                    nc.vector.reduce_max(mx[:], h3[:], mybir.AxisListType.X,
                                         negate=True)
                    mxb = mx[:].rearrange("p (m o) -> p m o", o=1) \
                               .broadcast_to((128, MT, F3))
                    ex = wpool.tile([128, MT, F3], F32, tag="ex")
                    nc.vector.tensor_tensor(ex[:], h3[:], mxb,
                                            mybir.AluOpType.add)
                    nc.scalar.activation(ex[:], ex[:],
                                         mybir.ActivationFunctionType.Exp)
                    sm = wpool.tile([128, MT], F32, tag="sm")
                    nc.vector.reduce_sum(sm[:], ex[:], mybir.AxisListType.X)
                    rc = wpool.tile([128, MT], F32, tag="rc")
                    nc.vector.reciprocal(rc[:], sm[:])
                    rcb = rc[:].rearrange("p (m o) -> p m o", o=1) \
                               .broadcast_to((128, MT, F3))
                    o_sb = wpool.tile([128, MT, F3], F32, tag="osm")
                    nc.vector.tensor_tensor(o_sb[:], ex[:], rcb,
                                            mybir.AluOpType.mult)
                    nc.sync.dma_start(
                        out=out_d.ap().rearrange("(p m) c -> p m c", p=128),
                        in_=o_sb[:])

    nc.compile()
    return nc


def _get_program():
    if "nc" not in _prog_cache:
        _prog_cache["nc"] = _build_program()
    return _prog_cache["nc"]


def _preprocess(x, edge_index, W1, b1, W2, b2, W3, b3):
    x = np.asarray(x, dtype=np.float32)
    ei = np.asarray(edge_index)
    row = ei[0].astype(np.int64)
    col = ei[1].astype(np.int64)

    deg = np.bincount(col, minlength=N).astype(np.float32) + 1.0
    dis = (1.0 / np.sqrt(deg)).astype(np.float32)

    # dense count matrix with self loops, exact small ints in fp8-e4m3
    A = np.zeros((N, N), dtype=np.uint8)
    np.add.at(A, (row, col), 1)
    idx = np.arange(N)
    A[idx, idx] += 1
    assert A.max() <= 16, "fp8 count matrix would be inexact"
    lut = (np.arange(256, dtype=np.uint8).astype(np.float32) / 64.0) \
        .astype(NP_FP8).view(np.uint8)
    A8 = lut[A]  # uint8 bit patterns of fp8 counts * 2^-6

    # source-row permutation: aggregation tile j = d*16+m holds, on
    # partition p, global node d*2048 + p*16 + m
    g = np.arange(N)
    jj, pp = g // 128, g % 128
    dd, mm = jj // MT, jj % MT
    perm_src = dd * CP + pp * MT + mm
    A8p = A8[perm_src, :]
    Abp = (A[perm_src, :] >= 1).astype(np.uint8)   # 0/1 indicator

    w1s = (np.asarray(W1, dtype=np.float32) * W1_SCALE).astype(NP_FP8)

    in_maps = []
    for d in range(NCORES):
        sl = slice(d * CP, (d + 1) * CP)
        dis_d = dis[sl]
        # partition-major pre-tiling (see kernel comments for layouts)
        xtp = x[sl, :].T.astype(NP_FP8)                # [16384 feat, 2048 n]
        xtp = xtp.reshape(KT // SB, SB, 128, CP) \
                 .transpose(2, 0, 1, 3).reshape(128, N * CP // 128)
        xt_parts = np.split(xtp, NXT, axis=1)
        a_sl = A8p[:, sl]                              # [16384 src, 2048 t]
        a_sl = a_sl.reshape(ST // SB, SB, 128, CP) \
                   .transpose(2, 0, 1, 3).reshape(128, N * CP // 128)
        ab_sl = Abp[:, sl].reshape(ST // SB, SB, 128, CP) \
                          .transpose(2, 0, 1, 3).reshape(128, ST * CP)
        astp = np.packbits(
            ab_sl[:, RES * CP:].reshape(128, (ST - RES) * CP // 8, 8),
            axis=-1, bitorder="little")[:, :, 0]
        m = {
            "ares_d": np.ascontiguousarray(a_sl[:, :RES * CP]).view(NP_FP8),
            "astp_d": np.ascontiguousarray(astp),
            "w1_d": np.ascontiguousarray(
                w1s.reshape(KT, 128, F1).transpose(1, 0, 2)
                   .reshape(128, KT * F1)),
            "w2_d": np.ascontiguousarray(W2, dtype=np.float32),
            "w3_d": np.ascontiguousarray(W3, dtype=np.float32),
            "b1_d": np.ascontiguousarray(b1, dtype=np.float32).reshape(F1, 1),
            "b2_d": np.ascontiguousarray(b2, dtype=np.float32).reshape(F2, 1),
            "b3_d": np.ascontiguousarray(b3, dtype=np.float32).reshape(F3, 1),
            "dis16_l1": np.ascontiguousarray(
                (dis_d * (ALPHA[1] / W1_SCALE)).reshape(128, MT)),
            "dis16_l2": np.ascontiguousarray(
                (dis_d * ALPHA[2]).reshape(128, MT)),
            "dis16_l3": np.ascontiguousarray(
                (dis_d * ALPHA[3]).reshape(128, MT)),
            "disrep_d": np.ascontiguousarray(
                np.broadcast_to(dis_d[None, :], (F1, CP))),
        }
        for i in range(NXT):
            m[f"xt{i}_d"] = np.ascontiguousarray(xt_parts[i])
        in_maps.append(m)
    return in_maps


def _execute(in_maps, trace=False, trace_cores=None):
    nc = _get_program()
    return run_bass_kernel_spmd(nc, in_maps,
                                core_ids=list(range(NCORES)), trace=trace,
                                trace_cores=trace_cores)


def kernel(x, edge_index, W1, b1, W2, b2, W3, b3):
    in_maps = _preprocess(x, edge_index, W1, b1, W2, b2, W3, b3)
    res = _execute(in_maps, trace=False)
    return np.concatenate([r["out_d"] for r in res.results], axis=0)


# revision 16
# speedup vs baseline: 1.4228x; 1.4228x over previous
"""3-layer GCN (gcn_norm + 3x gcn_conv + softmax) on 8 Trainium2 NeuronCores.

Strategy (self-contained; shapes hardcoded for N=16384, E=524288):
  - Node (row) sharding: core d owns nodes [d*2048, (d+1)*2048).
  - Everything big is fp8-e4m3 with power-of-2 scale factors folded into the
    per-node scale tensors, enabling the PE's DoubleRow perf mode (2 k-tiles
    of 128 per pass) for both the x@W1 stage and the dense aggregations.
  - Stage 1: P1 = x @ (W1*64) from a host-transposed fp8 x shard; the 1/64
    is folded into the dis scale applied afterwards.
  - Aggregation out = D^-1/2 (A + I) D^-1/2 (h W): normalization folded into
    per-node scales dis = deg^-1/2 applied before/after a plain *count*
    aggregation.  The count matrix (A + I) is dense fp8 per-core column shard
    [16384, 2048]; small-integer counts are exact in fp8.
  - The first 48 source-tiles of the count matrix (12 MiB) stay RESIDENT in
    SBUF across all three layers; only the remaining 20 MiB is re-streamed
    per layer.  Each layer consumes the resident part first so the PE can
    start right after the AllGather while the stream refills.
  - Features are stored as alpha_l * dis_n * (h W)_n in fp8; the epilogue
    multiplies by dis_t (one shared disrep) with 1/alpha_l folded into the
    activation's scale argument.
  - Per-core node shards of the fp8 features are AllGathered so every core
    holds the full source-side operand in SBUF.
  - Input DRAM tensors are declared in consumption order and the big ones
    split, so the runtime's input upload pipelines with compute.

kernel(**inputs) takes the FULL inputs and returns the FULL [16384, 16] fp32
output.
"""

import numpy as np

import concourse.bass as bass
import concourse.mybir as mybir
import concourse.tile as tile
from concourse import bacc
from concourse.bass_utils import run_bass_kernel_spmd
from concourse.masks import make_identity

N = 16384
NCORES = 8
CP = N // NCORES          # 2048 nodes per core
F1, F2, F3 = 64, 32, 16
KT = N // 128             # 128 k-tiles in stage 1
ST = N // 128             # 128 source tiles in aggregation
MT = CP // 128            # 16 m-tiles (local node tiles)
NCHUNK = CP // 512        # 4 free-dim chunks of 512
SB = 8                    # k/source tiles per 2 MiB stream DMA
RES = 48                  # resident source tiles (12 MiB SBUF)
NXT = 4                   # xt_d split into NXT input tensors

W1_SCALE = 64.0           # W1 pre-scaled into e4m3 normal range
ALPHA = {1: 4.0, 2: 32.0, 3: 128.0}   # fp8 feature scales per layer

F32 = mybir.dt.float32
F16 = mybir.dt.float16
FP8 = mybir.dt.float8e4
NP_FP8 = mybir.dt.np(FP8)
DR = mybir.MatmulPerfMode.DoubleRow

_prog_cache = {}


def _build_program():
    nc = bacc.Bacc("TRN2", target_bir_lowering=False, debug=False,
                   num_devices=NCORES)

    # Declaration order == consumption order (the runtime's input upload is
    # roughly in-order; the kernel's first reads then wait minimally).
    w1_d = nc.dram_tensor("w1_d", [128, KT * F1], FP8, kind="ExternalInput")
    w2_d = nc.dram_tensor("w2_d", [F1, F2], F32, kind="ExternalInput")
    w3_d = nc.dram_tensor("w3_d", [F2, F3], F32, kind="ExternalInput")
    b1_d = nc.dram_tensor("b1_d", [F1, 1], F32, kind="ExternalInput")
    b2_d = nc.dram_tensor("b2_d", [F2, 1], F32, kind="ExternalInput")
    b3_d = nc.dram_tensor("b3_d", [F3, 1], F32, kind="ExternalInput")
    dis16_l1 = nc.dram_tensor("dis16_l1", [128, MT], F32, kind="ExternalInput")
    dis16_l2 = nc.dram_tensor("dis16_l2", [128, MT], F32, kind="ExternalInput")
    dis16_l3 = nc.dram_tensor("dis16_l3", [128, MT], F32, kind="ExternalInput")
    disrep_d = nc.dram_tensor("disrep_d", [F1, CP], F32, kind="ExternalInput")
    # xt parts: [p, kb, b, t], feature = ((x*(KT//SB//NXT)+kb)*8+b)*128 + p
    xt_d = [nc.dram_tensor(f"xt{i}_d", [128, N * CP // 128 // NXT], FP8,
                           kind="ExternalInput") for i in range(NXT)]
    # resident part of the count matrix: source tiles 0..RES-1
    ares_d = nc.dram_tensor("ares_d", [128, RES * CP], FP8,
                            kind="ExternalInput")
    # streamed part: source tiles RES..127, [p, jb, bb, t] layout
    astr_d = nc.dram_tensor("astr_d", [128, (ST - RES) * CP], FP8,
                            kind="ExternalInput")
    out_d = nc.dram_tensor("out_d", [CP, F3], F32, kind="ExternalOutput")

    # DRAM bounce buffers for the AllGathers (in: Local, out: Shared)
    ps_in = {}
    ps_out = {}
    for li, F in ((1, F1), (2, F2), (3, F3)):
        ps_in[li] = nc.dram_tensor(f"ps_in_{li}", [CP, F], FP8)
        ps_out[li] = nc.dram_tensor(f"ps_out_{li}", [N, F], FP8,
                                    addr_space="Shared")

    with tile.TileContext(nc) as tc:
        with tc.tile_pool(name="const", bufs=1) as cpool, \
             tc.tile_pool(name="stream", bufs=4) as spool, \
             tc.tile_pool(name="ps", bufs=1) as pspool, \
             tc.tile_pool(name="work", bufs=1) as wpool, \
             tc.tile_pool(name="psum", bufs=2, space="PSUM") as psum, \
             tc.tile_pool(name="psum_acc", bufs=1, space="PSUM") as psum_acc:

            # ---- constants -------------------------------------------------
            w1_sb = cpool.tile([128, KT * F1], FP8, tag="w1")
            nc.scalar.dma_start(out=w1_sb[:], in_=w1_d[:, :])
            w2_sb = cpool.tile([F1, F2], F32, tag="w2")
            nc.scalar.dma_start(out=w2_sb[:], in_=w2_d[:, :])
            w3_sb = cpool.tile([F2, F3], F32, tag="w3")
            nc.scalar.dma_start(out=w3_sb[:], in_=w3_d[:, :])
            b_sb = {}
            for li, (bd, F) in ((1, (b1_d, F1)), (2, (b2_d, F2)), (3, (b3_d, F3))):
                b_sb[li] = cpool.tile([F, 1], F32, tag=f"b{li}", name=f"b{li}_sb")
                nc.scalar.dma_start(out=b_sb[li][:], in_=bd[:, :])
            dis16_sb = {}
            for li, dd in ((1, dis16_l1), (2, dis16_l2), (3, dis16_l3)):
                dis16_sb[li] = cpool.tile([128, MT], F32, tag=f"dis16_{li}",
                                          name=f"dis16_sb{li}")
                nc.scalar.dma_start(out=dis16_sb[li][:], in_=dd[:, :])
            disrep_sb = cpool.tile([F1, CP], F32, tag="disrep")
            nc.scalar.dma_start(out=disrep_sb[:], in_=disrep_d[:, :])
            ident = cpool.tile([128, 128], F32, tag="ident")
            make_identity(nc, ident[:])

            # resident count-matrix block, loaded once (6 x 2 MiB DMAs)
            ares_sb = cpool.tile([128, RES, CP], FP8, tag="ares")
            aresr = ares_sb[:]

            def strided_m(t, m):
                """[F, CP] tile -> [F, 128] slice holding nodes q*16+m."""
                return t[:].rearrange("f (q m) -> f m q", m=MT)[:, m, :]

            rings = [nc.sync, nc.scalar, nc.gpsimd]

            # ---- stage 1: P1T = (x @ W1)^T for local nodes ----------------
            w1r = w1_sb[:].rearrange("p (k f) -> p k f", k=KT)
            p1t_ps = [psum_acc.tile([F1, 512], F32, tag=f"acc{c}",
                                    name=f"p1t_ps{c}")
                      for c in range(NCHUNK)]
            KBT = KT // SB // NXT   # kb tiles per xt part
            for kb in range(KT // SB):
                xt_tile = spool.tile([128, SB, CP], FP8, tag="stream")
                off = (kb % KBT) * SB * CP
                rings[kb % 2].dma_start(
                    out=xt_tile[:],
                    in_=xt_d[kb // KBT][:, off:off + SB * CP]
                        .rearrange("p (b t) -> p b t", b=SB))
                # interleave the resident-A load on the third ring
                if kb % 2 == 0 and kb // 2 < RES // SB:
                    jb = kb // 2
                    nc.gpsimd.dma_start(
                        out=ares_sb[:, jb * SB:(jb + 1) * SB, :],
                        in_=ares_d[:, jb * SB * CP:(jb + 1) * SB * CP]
                            .rearrange("p (b t) -> p b t", b=SB))
                for bp in range(SB // 2):
                    k2 = kb * (SB // 2) + bp       # pair index, 0..63
                    for c in range(NCHUNK):
                        nc.tensor.matmul(
                            p1t_ps[c][:],
                            lhsT=w1r[:, 2 * k2:2 * k2 + 2, :],
                            rhs=xt_tile[:, 2 * bp:2 * bp + 2,
                                        c * 512:(c + 1) * 512],
                            start=(k2 == 0), stop=(k2 == KT // 2 - 1),
                            perf_mode=DR)
            p1t_sb = wpool.tile([F1, CP], F32, tag="hT")
            for c in range(NCHUNK):
                nc.vector.tensor_copy(p1t_sb[:, c * 512:(c + 1) * 512],
                                      p1t_ps[c][:])
            ps_local1 = wpool.tile([128, MT, F1], FP8, tag="psl1")
            for m in range(MT):
                pt = psum.tile([128, F1], F32, tag="wmul")
                nc.tensor.transpose(pt[:], strided_m(p1t_sb, m),
                                    ident[:F1, :F1])
                nc.vector.tensor_scalar_mul(ps_local1[:, m, :], pt[:],
                                            dis16_sb[1][:, m:m + 1])
            nc.sync.dma_start(
                out=ps_in[1].ap().rearrange("(p m) f -> p m f", p=128),
                in_=ps_local1[:])
            nc.gpsimd.collective_compute(
                "AllGather", mybir.AluOpType.bypass,
                replica_groups=[list(range(NCORES))],
                ins=[ps_in[1].ap().opt()],
                outs=[ps_out[1].ap().opt()],
            )

            def emit_wmul_scale_gather(hT_sb, F_in, F_nxt, w_sb, li):
                """normal-land W-mul + dis scale + fp8 cast, per m-tile with
                node-to-(partition, m) mapping n = p*16 + m; then bounce to
                DRAM and AllGather."""
                ps_local = wpool.tile([128, MT, F_nxt], FP8, tag=f"psl{li}")
                for m in range(MT):
                    pt = psum.tile([128, F_nxt], F32, tag="wmul")
                    nc.tensor.matmul(pt[:], lhsT=strided_m(hT_sb, m),
                                     rhs=w_sb[:], start=True, stop=True)
                    nc.vector.tensor_scalar_mul(ps_local[:, m, :], pt[:],
                                                dis16_sb[li][:, m:m + 1])
                nc.sync.dma_start(
                    out=ps_in[li].ap().rearrange("(p m) f -> p m f", p=128),
                    in_=ps_local[:])
                nc.gpsimd.collective_compute(
                    "AllGather", mybir.AluOpType.bypass,
                    replica_groups=[list(range(NCORES))],
                    ins=[ps_in[li].ap().opt()],
                    outs=[ps_out[li].ap().opt()],
                )
                ps_full = pspool.tile([128, ST * F_nxt], FP8, tag=f"psf{li}")
                # dram row of (p, d, m) is d*2048 + p*16 + m; j = d*16 + m
                nc.sync.dma_start(
                    out=ps_full[:].rearrange("p (d m f) -> p d m f",
                                             d=NCORES, m=MT),
                    in_=ps_out[li].ap().rearrange("(d p m) f -> p d m f",
                                                  d=NCORES, p=128))
                return ps_full

            ps_full = pspool.tile([128, ST * F1], FP8, tag="psf1")
            nc.scalar.dma_start(
                out=ps_full[:].rearrange("p (d m f) -> p d m f",
                                         d=NCORES, m=MT),
                in_=ps_out[1].ap().rearrange("(d p m) f -> p d m f",
                                             d=NCORES, p=128))

            # ---- layers ----------------------------------------------------
            for li, F in ((1, F1), (2, F2), (3, F3)):
                psfr = ps_full[:].rearrange("p (j f) -> p j f", j=ST)
                agg_ps = [psum_acc.tile([F, 512], F32, tag=f"acc{c}",
                                        name=f"agg{li}_ps{c}")
                          for c in range(NCHUNK)]

                def pair_mm(j2, rhs_ap, start, stop):
                    for c in range(NCHUNK):
                        nc.tensor.matmul(
                            agg_ps[c][:],
                            lhsT=psfr[:, 2 * j2:2 * j2 + 2, :],
                            rhs=rhs_ap[:, :, c * 512:(c + 1) * 512],
                            start=start, stop=stop, perf_mode=DR)

                # resident source tiles first: PE starts right after the
                # AllGather while the stream part is still in flight
                for j2 in range(RES // 2):
                    pair_mm(j2, aresr[:, 2 * j2:2 * j2 + 2, :],
                            start=(j2 == 0), stop=False)
                # streamed source tiles RES..127
                for jb in range((ST - RES) // SB):
                    a_tile = spool.tile([128, SB, CP], FP8, tag="stream")
                    eng = rings[jb % 2]  # keep streams off gpsimd: it hosts the collectives
                    eng.dma_start(
                        out=a_tile[:],
                        in_=astr_d[:, jb * SB * CP:(jb + 1) * SB * CP]
                            .rearrange("p (b t) -> p b t", b=SB))
                    for bp in range(SB // 2):
                        j2 = RES // 2 + jb * (SB // 2) + bp
                        pair_mm(j2, a_tile[:, 2 * bp:2 * bp + 2, :],
                                start=False, stop=(j2 == ST // 2 - 1))

                hT_sb = wpool.tile([F, CP], F32, tag="hT")
                for c in range(NCHUNK):
                    sl = slice(c * 512, (c + 1) * 512)
                    zt = wpool.tile([F, 512], F32, tag="zt")
                    nc.vector.tensor_tensor(zt[:], agg_ps[c][:],
                                            disrep_sb[:F, sl],
                                            mybir.AluOpType.mult)
                    func = (mybir.ActivationFunctionType.Relu if li < 3
                            else mybir.ActivationFunctionType.Identity)
                    nc.scalar.activation(hT_sb[:, sl], zt[:], func,
                                         bias=b_sb[li][:, 0:1],
                                         scale=1.0 / ALPHA[li])
                if li == 1:
                    ps_full = emit_wmul_scale_gather(hT_sb, F1, F2, w2_sb, 2)
                elif li == 2:
                    ps_full = emit_wmul_scale_gather(hT_sb, F2, F3, w3_sb, 3)
                else:
                    # transpose + batched softmax over classes (free dim)
                    h3 = wpool.tile([128, MT, F3], F32, tag="h3")
                    for m in range(MT):
                        pt = psum.tile([128, F3], F32, tag="wmul")
                        nc.tensor.transpose(pt[:], strided_m(hT_sb, m),
                                            ident[:F3, :F3])
                        nc.vector.tensor_copy(h3[:, m, :], pt[:])
                    mx = wpool.tile([128, MT], F32, tag="mx")
                    nc.vector.reduce_max(mx[:], h3[:], mybir.AxisListType.X,
                                         negate=True)
                    mxb = mx[:].rearrange("p (m o) -> p m o", o=1) \
                               .broadcast_to((128, MT, F3))
                    ex = wpool.tile([128, MT, F3], F32, tag="ex")
                    nc.vector.tensor_tensor(ex[:], h3[:], mxb,
                                            mybir.AluOpType.add)
                    nc.scalar.activation(ex[:], ex[:],
                                         mybir.ActivationFunctionType.Exp)
                    sm = wpool.tile([128, MT], F32, tag="sm")
                    nc.vector.reduce_sum(sm[:], ex[:], mybir.AxisListType.X)
                    rc = wpool.tile([128, MT], F32, tag="rc")
                    nc.vector.reciprocal(rc[:], sm[:])
                    rcb = rc[:].rearrange("p (m o) -> p m o", o=1) \
                               .broadcast_to((128, MT, F3))
                    o_sb = wpool.tile([128, MT, F3], F32, tag="osm")
                    nc.vector.tensor_tensor(o_sb[:], ex[:], rcb,
                                            mybir.AluOpType.mult)
                    nc.sync.dma_start(
                        out=out_d.ap().rearrange("(p m) c -> p m c", p=128),
                        in_=o_sb[:])

    nc.compile()
    return nc


def _get_program():
    if "nc" not in _prog_cache:
        _prog_cache["nc"] = _build_program()
    return _prog_cache["nc"]


def _preprocess(x, edge_index, W1, b1, W2, b2, W3, b3):
    x = np.asarray(x, dtype=np.float32)
    ei = np.asarray(edge_index)
    row = ei[0].astype(np.int64)
    col = ei[1].astype(np.int64)

    deg = np.bincount(col, minlength=N).astype(np.float32) + 1.0
    dis = (1.0 / np.sqrt(deg)).astype(np.float32)

    # dense count matrix with self loops, exact small ints in fp8-e4m3
    A = np.zeros((N, N), dtype=np.uint8)
    np.add.at(A, (row, col), 1)
    idx = np.arange(N)
    A[idx, idx] += 1
    assert A.max() <= 16, "fp8 count matrix would be inexact"
    lut = np.arange(256, dtype=np.uint8).astype(np.float32) \
            .astype(NP_FP8).view(np.uint8)
    A8 = lut[A]  # uint8 bit patterns of fp8 counts

    # source-row permutation: aggregation tile j = d*16+m holds, on
    # partition p, global node d*2048 + p*16 + m
    g = np.arange(N)
    jj, pp = g // 128, g % 128
    dd, mm = jj // MT, jj % MT
    perm_src = dd * CP + pp * MT + mm
    A8p = A8[perm_src, :]

    w1s = (np.asarray(W1, dtype=np.float32) * W1_SCALE).astype(NP_FP8)

    in_maps = []
    for d in range(NCORES):
        sl = slice(d * CP, (d + 1) * CP)
        dis_d = dis[sl]
        # partition-major pre-tiling (see kernel comments for layouts)
        xtp = x[sl, :].T.astype(NP_FP8)                # [16384 feat, 2048 n]
        xtp = xtp.reshape(KT // SB, SB, 128, CP) \
                 .transpose(2, 0, 1, 3).reshape(128, N * CP // 128)
        xt_parts = np.split(xtp, NXT, axis=1)
        a_sl = A8p[:, sl]                              # [16384 src, 2048 t]
        a_sl = a_sl.reshape(ST // SB, SB, 128, CP) \
                   .transpose(2, 0, 1, 3).reshape(128, N * CP // 128)
        m = {
            "ares_d": np.ascontiguousarray(a_sl[:, :RES * CP]).view(NP_FP8),
            "astr_d": np.ascontiguousarray(a_sl[:, RES * CP:]).view(NP_FP8),
            "w1_d": np.ascontiguousarray(
                w1s.reshape(KT, 128, F1).transpose(1, 0, 2)
                   .reshape(128, KT * F1)),
            "w2_d": np.ascontiguousarray(W2, dtype=np.float32),
            "w3_d": np.ascontiguousarray(W3, dtype=np.float32),
            "b1_d": np.ascontiguousarray(b1, dtype=np.float32).reshape(F1, 1),
            "b2_d": np.ascontiguousarray(b2, dtype=np.float32).reshape(F2, 1),
            "b3_d": np.ascontiguousarray(b3, dtype=np.float32).reshape(F3, 1),
            "dis16_l1": np.ascontiguousarray(
                (dis_d * (ALPHA[1] / W1_SCALE)).reshape(128, MT)),
            "dis16_l2": np.ascontiguousarray(
                (dis_d * ALPHA[2]).reshape(128, MT)),
            "dis16_l3": np.ascontiguousarray(
                (dis_d * ALPHA[3]).reshape(128, MT)),
            "disrep_d": np.ascontiguousarray(
                np.broadcast_to(dis_d[None, :], (F1, CP))),
        }
        for i in range(NXT):
            m[f"xt{i}_d"] = np.ascontiguousarray(xt_parts[i])
        in_maps.append(m)
    return in_maps


def _execute(in_maps, trace=False, trace_cores=None):
    nc = _get_program()
    return run_bass_kernel_spmd(nc, in_maps,
                                core_ids=list(range(NCORES)), trace=trace,
                                trace_cores=trace_cores)


def kernel(x, edge_index, W1, b1, W2, b2, W3, b3):
    in_maps = _preprocess(x, edge_index, W1, b1, W2, b2, W3, b3)
    res = _execute(in_maps, trace=False)
    return np.concatenate([r["out_d"] for r in res.results], axis=0)
